# revision 45
# baseline (speedup 1.0000x reference)
"""Binarized 2-layer conv net (BinaryConv2d -> BinaryTanh -> BinaryConv2d -> Scale)
for Trainium2, data-parallel over the batch dim across 8 NeuronCores.

Math (matching the reference):
    h   = conv2d(x, sign(w1), pad=1) + sign(b1)
    h   = sign(h)                       # sign(clip(h,-1,1)) == sign(h)
    out = (conv2d(h, sign(w2), pad=1) + sign(b2)) * scale

Device mapping (per core, 8 images):
  * x split on host into fp16 hi + fp16 lo (~24 mantissa bits together,
    effectively fp32-exact), pre-padded to 66x66.
  * x load: 4 DMAs of 32 descriptors per image build a [128, 66*66]
    buffer whose partitions are [hi grp0, hi grp1, lo grp0, lo grp1];
    grp1 is grp0 shifted one padded row (overlapping strided src APs),
    so one K=128 matmul covers taps (0,dx)+(1,dx) for BOTH precisions.
  * conv1: 4 rounds x 6 passes x 2 PE col tiles, K=128. Pass pt=0
    covers taps (0,dx)+(1,dx) hi+lo; pt=1 covers (2,dx) with the grp1
    rows zero-weighted. Round r computes g0 block r (col tile 0) and
    g1 block r+4 (col tile 1) into one [128,512] PSUM bank;
    sign(conv1+b1) evacuates in a single full-lane ScalarE ACT.
    Rounds run in order (3,0,1,2) so both halo source rows exist as
    early as possible.
  * h layout: big contiguous slab [128, 34*66] bf16. Partitions 0:64
    (g0) hold image rows -1..32 as slab rows 0..33; partitions 64:128
    (g1) hold rows 31..64. Two tiny halo DMAs per image (scalar-
    triggered rings): g0 row 33 <- g1 row 1 after round 0, g1 row 0 <-
    g0 row 32 after round 3.
  * conv2: bf16, K=64, 4 concurrent 64x64 PE tiles (2 row groups x 2
    col groups) process 4 blocks at once; 9 tap-matmuls each; dy
    orders put halo-dependent taps last. PSUM evac (out = psum +
    sign(b2), exact small ints in bf16) via two DVE tensor_scalar ops
    into a [128, 2048] staging buffer; 2 out DMAs per image.
  * DMA lessons baked in: HW-DGE descriptor generation runs on the
    triggering engine and a trigger occupies the shared DMA-engine
    pool for its whole transfer (~197 GB/s/core measured), so bulk
    transfers are split into <=32-descriptor chunks (short pool holds
    let the latency-critical halo DMAs through), bulk x/out triggers
    live on the sync queue (x prefetched 2 images ahead, nbuf=3), and
    the scalar queue runs only conv1 ACTs + halo triggers. Out HBM
    layout is partition-major [128, 8, 2048] (host reorders to NCHW).
  * A 16-matmul zero warmup runs during the first x load so the PE
    HAM clock is at 8/8 when real work starts.
"""

import numpy as np
import ml_dtypes

import bass_rust
import concourse.bass as bass
import concourse.mybir as mybir
import concourse.tile as tile
from concourse import bacc
from concourse.bass_utils import run_bass_kernel_spmd

F32 = mybir.dt.float32
F16 = mybir.dt.float16
BF16 = mybir.dt.bfloat16

N_CORES = 8
IMGS_PER_CORE = 8
CIN, COUT = 32, 64
H = W = 64
WP = 66                    # padded width
T1_FREE = H * WP           # 4224, one dy-block slab
HS_ROWS = 34               # h slab rows per partition group (img rows -1..32 / 31..64)
HS_FREE = HS_ROWS * WP     # 2244
ACT_SIGN = mybir.ActivationFunctionType.Sign
ACT_IDENT = mybir.ActivationFunctionType.Identity
ALU_ADD = mybir.AluOpType.add


def _raw_ap(tensor_handle, offset, dims):
    """Build an AP with explicit [stride, size] dims (allows overlap)."""
    return bass_rust.AP(tensor=tensor_handle, offset=offset,
                        ap=[[s, n] for s, n in dims])


def build_nc(reps: int = 1, nbuf: int = 3, pa_bufs: int = 2,
             hbufs: int = 2) -> bacc.Bacc:
    nc = bacc.Bacc("TRN2", target_bir_lowering=False)

    # xin: flat [img][prec][c][66*66] fp16 (hi then lo, both host-padded),
    # plus 66 elements of slack for the shifted group-1 reads of the last
    # image (they run past the tensor by up to 65 elements, zero-weighted).
    XIN_IMG = 2 * CIN * WP * WP
    xin_t = nc.dram_tensor("xin", [IMGS_PER_CORE * XIN_IMG + WP], F16, kind="ExternalInput")
    w1s_t = nc.dram_tensor("w1s", [128, 384], F16, kind="ExternalInput")
    w2s_t = nc.dram_tensor("w2s", [128, 576], BF16, kind="ExternalInput")
    b1s_t = nc.dram_tensor("b1s", [128, 1], F32, kind="ExternalInput")
    b2s_t = nc.dram_tensor("b2s", [128, 1], F32, kind="ExternalInput")
    # partition-major output: [128 sbuf partitions, img, (g, s2, px)]
    out_t = nc.dram_tensor("out", [128, IMGS_PER_CORE, 2048], BF16, kind="ExternalOutput")

    xin_h = xin_t.ap().tensor
    outr = out_t.ap()

    with tile.TileContext(nc) as tc:
        # ---- persistent SBUF tensors ----
        w1 = nc.alloc_sbuf_tensor("w1sb", [128, 384], F16).ap()
        w2 = nc.alloc_sbuf_tensor("w2sb", [128, 576], BF16).ap()
        b1 = nc.alloc_sbuf_tensor("b1sb", [128, 1], F32).ap()
        b2 = nc.alloc_sbuf_tensor("b2sb", [128, 1], F32).ap()
        # x buffer: partitions = [hi grp0, hi grp1, lo grp0, lo grp1], where
        # grp0 holds x_pad[c] rows 0..65 and grp1 the same shifted one row
        # (tap dy0 and dy1 of any row land at the same free offset).
        xb = [nc.alloc_sbuf_tensor(f"xb_{b}", [128, WP * WP], F16).ap()
              for b in range(nbuf)]
        hs = [nc.alloc_sbuf_tensor(f"hs_{b}", [128, HS_FREE], BF16).ap()
              for b in range(hbufs)]
        ob = [nc.alloc_sbuf_tensor(f"ob_{b}", [128, 2048], BF16).ap()
              for b in range(2)]


        # h slab borders (col 0/65 of every row, g0 row 0, g1 row 33) must
        # stay zero forever; per-image writes only touch interior cols 1:65
        # of rows 1..32 plus the two halo rows.
        for b in range(hbufs):
            nc.gpsimd.memset(hs[b][:, :], 0.0)
        # HAM warmup scratch: zeros, memset on DVE so it's ready ~7us in.
        wz = nc.alloc_sbuf_tensor("wzsb", [128, 512], BF16).ap()
        nc.vector.memset(wz[:, :], 0.0)

        # conv2 tap orders: boundary taps last.
        # pc (g0): block 3 dy=2 reads img row 32, which lives in g1 row 1.
        # pd (g1): block 4 dy=0 reads img row 31 (g0 row 32, round-3 ACT).
        TAPS_PC = [(dy, dx) for dy in (0, 1, 2) for dx in range(3)]
        TAPS_PD = [(dy, dx) for dy in (1, 2, 0) for dx in range(3)]

        with tc.tile_pool(name="psA", bufs=pa_bufs, space="PSUM") as pool_a, \
             tc.tile_pool(name="psB", bufs=2, space="PSUM") as pool_cd, \
             tc.tile_pool(name="psW", bufs=1, space="PSUM") as pool_w:
            n_iters = IMGS_PER_CORE * reps

            # HAM warmup: zero matmuls while the first x load is in
            # flight, so the PE clock is at 8/8 when real work starts.
            pw = pool_w.tile([128, 512], F32, tag="pw")
            for i in range(12):
                nc.tensor.matmul(pw[0:64, :], wz[:, 0:64], wz[:, :],
                                 start=(i == 0), stop=(i == 11),
                                 tile_position=(0, 0), skip_group_check=True)
            # keep the warmup alive past DCE (read pw once)
            nc.vector.tensor_scalar(out=wz[0:1, 0:1], in0=pw[0:1, 0:1],
                                    scalar1=0.0, scalar2=None, op0=ALU_ADD)

            def issue_x_part(iv, prec, grp, eng):
                im = iv % IMGS_PER_CORE
                src_ap = _raw_ap(
                    xin_h, im * XIN_IMG + prec * (CIN * WP * WP) + grp * WP,
                    [(WP * WP, CIN), (1, WP * WP)])
                p0 = prec * 64 + grp * 32
                eng.dma_start(out=xb[iv % nbuf][p0:p0 + 32, :], in_=src_ap)

            def issue_x_load(iv, split=False):
                # 4 DMAs of 32 descriptors each (one per precision x grp):
                # short DMA-engine-pool holds so tiny halo DMAs can
                # interleave. grp1 overlaps grp0 shifted one row (66 elems).
                # split=True (startup only): put half on the idle scalar
                # queue so trigger issue isn't serialized on sync.
                im = iv % IMGS_PER_CORE
                for prec in range(2):
                    for grp in range(2):
                        src = _raw_ap(
                            xin_h,
                            im * XIN_IMG + prec * (CIN * WP * WP) + grp * WP,
                            [(WP * WP, CIN), (1, WP * WP)])
                        p0 = prec * 64 + grp * 32
                        eng = nc.scalar if (split and grp == 1) else nc.sync
                        eng.dma_start(
                            out=xb[iv % nbuf][p0:p0 + 32, :], in_=src)

            # prefetch depth 2. ALL of x0 first on the sync queue (the
            # scalar queue's preamble can straggle, delaying its
            # triggers), then weights, then x1.
            issue_x_load(0)
            nc.sync.dma_start(out=w1, in_=w1s_t.ap())
            nc.scalar.dma_start(out=b1, in_=b1s_t.ap())
            nc.sync.dma_start(out=w2, in_=w2s_t.ap())
            nc.scalar.dma_start(out=b2, in_=b2s_t.ap())
            if n_iters > 1:
                issue_x_load(1)

            for img_v in range(n_iters):
                img = img_v % IMGS_PER_CORE
                tb = img_v % nbuf
                hb = img_v % hbufs
                if img_v + 2 < n_iters:
                    issue_x_load(img_v + 2)

                tv = xb[tb].rearrange("p (h w) -> p h w", w=WP)
                hv = hs[hb].rearrange("p (r w) -> p r w", w=WP)

                # ---- conv1: 4 rounds x (2 col tiles), K=128 ----
                # pass m = pt*3+dx: pt=0 covers taps (0,dx)+(1,dx) for hi+lo
                # (row offset 8R), pt=1 covers (2,dx) (row offset 8R+2, grp1
                # rows zero-weighted).
                for r in (3, 0, 1, 2):
                    pa = pool_a.tile([128, 512], F32, tag="pa")
                    n_mm = 0
                    for pt in range(2):
                        for dx in range(3):
                            lw = w1[0:128, (pt * 3 + dx) * 64:(pt * 3 + dx + 1) * 64]
                            o = 2 * pt
                            st = n_mm == 0
                            sp = n_mm == 5
                            nc.tensor.matmul(
                                pa[0:64, :], lw,
                                tv[0:128, 8 * r + o: 8 * r + o + 8, dx: dx + 64],
                                start=st, stop=sp, tile_position=(0, 0), skip_group_check=True)
                            nc.tensor.matmul(
                                pa[64:128, :], lw,
                                tv[0:128, 32 + 8 * r + o: 40 + 8 * r + o, dx: dx + 64],
                                start=st, stop=sp, tile_position=(0, 64), skip_group_check=True)
                            n_mm += 1
                    # h = sign(conv1 + b1): g0 block r / g1 block r+4 land at
                    # the same slab-row offsets -> one full-lane ACT.
                    nc.scalar.activation(
                        out=hv[:, 1 + 8 * r: 9 + 8 * r, 1:65],
                        in_=pa[:, :].rearrange("p (a b) -> p a b", b=64),
                        func=ACT_SIGN, bias=b1[:, 0:1])
                    # tiny cross-group halo copies on the scalar-triggered
                    # rings; round order (3,0,1,2) fires both as early as
                    # possible so their DMA-pool latency is fully hidden.
                    if r == 0:
                        nc.scalar.dma_start(out=hv[0:64, 33:34, 1:65],
                                            in_=hv[64:128, 1:2, 1:65])
                    if r == 3:
                        nc.scalar.dma_start(out=hv[64:128, 0:1, 1:65],
                                            in_=hv[0:64, 32:33, 1:65])


                # ---- conv2: 2 super-rounds x 4 concurrent 64x64 tiles ----
                obt = ob[img_v % 2]
                for s2 in range(2):
                    pc = pool_cd.tile([128, 512], F32, tag="pc")
                    pd = pool_cd.tile([128, 512], F32, tag="pd")
                    bA, bB = 2 * s2, 2 * s2 + 1
                    for ti in range(9):
                        st = ti == 0
                        sp = ti == 8
                        dyc, dxc = TAPS_PC[ti]
                        dyd, dxd = TAPS_PD[ti]
                        lwc = w2[0:64, (dyc * 3 + dxc) * 64:(dyc * 3 + dxc + 1) * 64]
                        lwd = w2[64:128, (dyd * 3 + dxd) * 64:(dyd * 3 + dxd + 1) * 64]
                        nc.tensor.matmul(
                            pc[0:64, :], lwc,
                            hv[0:64, 8 * bA + dyc: 8 * bA + dyc + 8, dxc: dxc + 64],
                            start=st, stop=sp, tile_position=(0, 0), skip_group_check=True)
                        nc.tensor.matmul(
                            pc[64:128, :], lwc,
                            hv[0:64, 8 * bB + dyc: 8 * bB + dyc + 8, dxc: dxc + 64],
                            start=st, stop=sp, tile_position=(0, 64), skip_group_check=True)
                        nc.tensor.matmul(
                            pd[0:64, :], lwd,
                            hv[64:128, 8 * bA + dyd: 8 * bA + dyd + 8, dxd: dxd + 64],
                            start=st, stop=sp, tile_position=(64, 0), skip_group_check=True)
                        nc.tensor.matmul(
                            pd[64:128, :], lwd,
                            hv[64:128, 8 * bB + dyd: 8 * bB + dyd + 8, dxd: dxd + 64],
                            start=st, stop=sp, tile_position=(64, 64), skip_group_check=True)
                    # out = psum + sign(b2), both halves on DVE.
                    # obt free layout: [s2=2 (1024), g=2 (512), px (1)]
                    nc.vector.tensor_scalar(
                        out=obt[:, s2 * 1024: s2 * 1024 + 512], in0=pc[:, :],
                        scalar1=b2[:, 0:1], scalar2=None, op0=ALU_ADD)
                    nc.vector.tensor_scalar(
                        out=obt[:, s2 * 1024 + 512: s2 * 1024 + 1024],
                        in0=pd[:, :],
                        scalar1=b2[:, 0:1], scalar2=None, op0=ALU_ADD)
                    if s2 == 0 and img_v == n_iters - 1:
                        # last image: drain the s2=0 half early (overlaps
                        # s2=1 compute) so the tail only ships 256KB.
                        nc.scalar.dma_start(out=outr[:, img, 0:1024],
                                            in_=obt[:, 0:1024])

                # ---- out DMA per image (sync queue, 2 x 64 descs) ----
                if img_v == n_iters - 1:
                    nc.sync.dma_start(out=outr[:, img, 1024:2048],
                                      in_=obt[:, 1024:2048])
                else:
                    nc.sync.dma_start(out=outr[0:64, img, :], in_=obt[0:64, :])
                    nc.sync.dma_start(out=outr[64:128, img, :], in_=obt[64:128, :])

    nc.compile()
    return nc


_CACHE: dict = {}


def _get_nc(reps: int = 1, **kw) -> bacc.Bacc:
    key = (reps, tuple(sorted(kw.items())))
    if key not in _CACHE:
        _CACHE[key] = build_nc(reps, **kw)
    return _CACHE[key]


def _sign(a: np.ndarray) -> np.ndarray:
    return np.where(a >= 0, np.float32(1.0), np.float32(-1.0))


def _prep_inputs(x, w1, b1, w2, b2, scale_val):
    x = np.asarray(x, np.float32)
    n = x.shape[0]
    # fp16 hi/lo split (hi+lo carries ~24 mantissa bits of x)
    xhi = x.astype(np.float16)
    xlo = (x - xhi.astype(np.float32)).astype(np.float16)
    xhi_pad = np.zeros((n, CIN, WP, WP), np.float16)
    xlo_pad = np.zeros((n, CIN, WP, WP), np.float16)
    xhi_pad[:, :, 1:65, 1:65] = xhi
    xlo_pad[:, :, 1:65, 1:65] = xlo

    per = n // N_CORES
    XIN_IMG = 2 * CIN * WP * WP
    xins = []
    for i in range(N_CORES):
        xin = np.zeros((IMGS_PER_CORE * XIN_IMG + WP,), np.float16)
        for j in range(IMGS_PER_CORE):
            im = i * per + j
            base = j * XIN_IMG
            xin[base:base + XIN_IMG // 2] = xhi_pad[im].reshape(-1)
            xin[base + XIN_IMG // 2:base + XIN_IMG] = xlo_pad[im].reshape(-1)
        xins.append(xin)

    w1b = _sign(np.asarray(w1, np.float32))           # [64o, 32c, 3, 3]
    w2b = _sign(np.asarray(w2, np.float32))           # [64o, 64c, 3, 3]
    # w1s [128, 6*64]: pass m = pt*3+dx; partition rows =
    # [hi grp0 (dy=pt*2? no: pt0->dy0), hi grp1 (dy1), lo grp0, lo grp1]
    w1s = np.zeros((128, 384), np.float16)
    for dx in range(3):
        # pt=0: taps (0,dx) on grp0 rows, (1,dx) on grp1 rows, both precisions
        w1s[0:32, dx * 64:(dx + 1) * 64] = w1b[:, :, 0, dx].T
        w1s[32:64, dx * 64:(dx + 1) * 64] = w1b[:, :, 1, dx].T
        w1s[64:96, dx * 64:(dx + 1) * 64] = w1b[:, :, 0, dx].T
        w1s[96:128, dx * 64:(dx + 1) * 64] = w1b[:, :, 1, dx].T
        # pt=1: tap (2,dx) on grp0 rows only; grp1 rows stay zero
        w1s[0:32, (3 + dx) * 64:(4 + dx) * 64] = w1b[:, :, 2, dx].T
        w1s[64:96, (3 + dx) * 64:(4 + dx) * 64] = w1b[:, :, 2, dx].T
    w2s = np.zeros((128, 576), ml_dtypes.bfloat16)
    for dy in range(3):
        for dx in range(3):
            tap = dy * 3 + dx
            blk = w2b[:, :, dy, dx].T.astype(ml_dtypes.bfloat16)
            w2s[0:64, tap * 64:(tap + 1) * 64] = blk
            w2s[64:128, tap * 64:(tap + 1) * 64] = blk
    b1s = np.tile(_sign(np.asarray(b1, np.float32)), 2).reshape(128, 1).astype(np.float32)
    b2s = np.tile(_sign(np.asarray(b2, np.float32)), 2).reshape(128, 1).astype(np.float32)

    in_maps = []
    for i in range(N_CORES):
        in_maps.append({
            "xin": xins[i],
            "w1s": w1s, "w2s": w2s, "b1s": b1s, "b2s": b2s,
        })
    return in_maps


def kernel(x, w1, b1, w2, b2, scale) -> np.ndarray:
    scale_val = float(np.asarray(scale).reshape(-1)[0])
    nc = _get_nc(reps=1)
    in_maps = _prep_inputs(x, w1, b1, w2, b2, scale_val)
    res = run_bass_kernel_spmd(nc, in_maps, core_ids=list(range(N_CORES)))
    # out HBM layout: [128, img, 2048] where partition p = (p_hi, ch),
    # free = (s, g, px); image block index = g*4 + s*2 + p_hi, image
    # pixel = block*512 + px (row-major 64x64).
    parts = []
    for r in res.results:
        o = np.asarray(r["out"]).reshape(2, 64, IMGS_PER_CORE, 2, 2, 512)
        o = o.transpose(2, 1, 4, 3, 0, 5).reshape(IMGS_PER_CORE, COUT, H, W)
        parts.append(o)
    out = np.concatenate(parts, axis=0)
    return out.astype(np.float32) * np.float32(scale_val)


if __name__ == "__main__":
    rng = np.random.default_rng(0)
    ins = {
        "x": rng.standard_normal((64, 32, 64, 64), dtype=np.float32),
        "w1": (rng.standard_normal((64, 32, 3, 3)) * 0.05).astype(np.float32),
        "b1": (rng.standard_normal((64,)) * 0.05).astype(np.float32),
        "w2": (rng.standard_normal((64, 64, 3, 3)) * 0.05).astype(np.float32),
        "b2": (rng.standard_normal((64,)) * 0.05).astype(np.float32),
        "scale": np.array([0.001], np.float32),
    }
    out = kernel(**ins)
    print("out", out.shape, out.dtype, float(np.abs(out).mean()))
